# revision 6
# baseline (speedup 1.0000x reference)
"""Multi-head attention Bass/Tile kernel for Trainium2, SPMD over 8 NeuronCores.

Problem: B=16, S=1024, D=512, H=8, dk=64.  out = MHA(x); returns
(out [B,S,D], attn_weights [B,H,S,S]).  Data-parallel over B: each of the 8
cores processes 2 batches end-to-end (no collectives needed).

Layout strategy per core (BL=2 local batches):
  xT   [din=128p, 4t, S]      <- PE-transpose of x_b
  QT/KT[dout=128p, 4t, S]     = W.T @ x.T   (head h lives at partitions
                                 64*(h%2) of tile h//2)
  V    [tok=128p, 8t, D]      = x @ Wv  (natural layout; lhsT for ctx matmul)
  per (b,h), per q-tile(128):
    scores psum [128q, 1024k] = QT_h.T @ KT_h   (K=dk=64)
    E = exp(scores/8)  (ACT, accum_out -> row sums)
    attn = E * recip(sums)   (DVE per-partition scalar) -> DMA out
    attnT [k=128p, 8t, 512q] <- PE-transpose of attn tiles
  per q-chunk(512): ctxT[dk,q] psum += V_h.T @ attnT   (K=k tiles)
  out[q, D] = sum_t ctxT_t.T @ Wo_t + bo  -> DMA out
"""

import os
import sys
import tempfile

import numpy as np

sys.path.insert(0, "/opt/trn_rl_repo")

B, S, D_MODEL, N_HEADS = 16, 1024, 512, 8
D_K = D_MODEL // N_HEADS        # 64
N_CORES = 8
BL = B // N_CORES               # 2 local batches per core
P = 128                         # partitions
DIN_T = D_MODEL // P            # 4 din tiles
QT_T = S // P                   # 8 q tiles per batch
KC = S // 512                   # 2 k chunks of 512
SCALE = 1.0 / float(np.sqrt(D_K))

# compute dtype for matmul operands: "float32" (exact, 4 cyc/row) or
# "float32r" (tf32-like, 1 cyc/row at free>=256)
MM_DTYPE = os.environ.get("MHA_MM_DTYPE", "float32r")


def _build(nc_holder=[]):
    import concourse.bass as bass
    import concourse.tile as tile
    from concourse import bacc, mybir
    from concourse.masks import make_identity

    fp32 = mybir.dt.float32
    cdt = getattr(mybir.dt, MM_DTYPE)
    AF = mybir.ActivationFunctionType

    nc = bacc.Bacc("TRN2", target_bir_lowering=False, debug=False,
                   num_devices=N_CORES)

    x = nc.dram_tensor("x", [BL, S, D_MODEL], fp32, kind="ExternalInput").ap()
    Wq = nc.dram_tensor("Wq", [D_MODEL, D_MODEL], fp32, kind="ExternalInput").ap()
    Wk = nc.dram_tensor("Wk", [D_MODEL, D_MODEL], fp32, kind="ExternalInput").ap()
    Wv = nc.dram_tensor("Wv", [D_MODEL, D_MODEL], fp32, kind="ExternalInput").ap()
    Wo = nc.dram_tensor("Wo", [D_MODEL, D_MODEL], fp32, kind="ExternalInput").ap()
    bq = nc.dram_tensor("bq", [D_MODEL], fp32, kind="ExternalInput").ap()
    bk = nc.dram_tensor("bk", [D_MODEL], fp32, kind="ExternalInput").ap()
    bv = nc.dram_tensor("bv", [D_MODEL], fp32, kind="ExternalInput").ap()
    bo = nc.dram_tensor("bo", [D_MODEL], fp32, kind="ExternalInput").ap()
    out = nc.dram_tensor("out", [BL, S, D_MODEL], fp32, kind="ExternalOutput").ap()
    attn = nc.dram_tensor("attn", [BL, N_HEADS, S, S], fp32,
                          kind="ExternalOutput").ap()

    from contextlib import ExitStack

    with tile.TileContext(nc) as tc, ExitStack() as ctx:
        const = ctx.enter_context(tc.tile_pool(name="const", bufs=1))
        bigp = ctx.enter_context(tc.tile_pool(name="bigp", bufs=1))   # per-batch 2MB tiles
        midp = ctx.enter_context(tc.tile_pool(name="midp", bufs=2))   # attnT ring
        rowp = ctx.enter_context(tc.tile_pool(name="rowp", bufs=3))   # E/attn rows
        smal = ctx.enter_context(tc.tile_pool(name="smal", bufs=4))   # stats
        outp = ctx.enter_context(tc.tile_pool(name="outp", bufs=3))
        # psum pools
        ps_mm = ctx.enter_context(tc.tile_pool(name="ps_mm", bufs=2, space="PSUM"))
        ps_tr = ctx.enter_context(tc.tile_pool(name="ps_tr", bufs=2, space="PSUM"))
        ps_cx = ctx.enter_context(tc.tile_pool(name="ps_cx", bufs=2, space="PSUM"))

        ident = const.tile([P, P], cdt)
        make_identity(nc, ident)

        ones = const.tile([1, P], cdt)
        nc.vector.memset(ones, 1.0)

        # weights in [din_part, din_tile, dout] layout, rounded to compute dtype
        def load_w(w_dram, name):
            raw = const.tile([P, DIN_T, D_MODEL], fp32, tag=f"{name}raw")
            nc.sync.dma_start(out=raw, in_=w_dram.rearrange("(t p) n -> p t n", p=P))
            if cdt == fp32:
                return raw
            r = const.tile([P, DIN_T, D_MODEL], cdt, tag=f"{name}r")
            nc.vector.tensor_copy(r, raw)
            return r

        Wq_sb = load_w(Wq, "wq")
        Wk_sb = load_w(Wk, "wk")
        Wv_sb = load_w(Wv, "wv")
        Wo_sb = load_w(Wo, "wo")

        # per-partition bias layout [128, 4] for Q/K (bias on dout partition dim)
        def load_b_part(b_dram, name):
            t = const.tile([P, DIN_T], fp32, tag=f"{name}p")
            nc.sync.dma_start(out=t, in_=b_dram.rearrange("(t p) -> p t", p=P))
            return t

        bq_sb = load_b_part(bq, "bq")
        bk_sb = load_b_part(bk, "bk")

        # row bias layout [1, 512] for V / out (bias on dout free dim)
        def load_b_row(b_dram, name):
            raw = const.tile([1, D_MODEL], fp32, tag=f"{name}rraw")
            nc.sync.dma_start(out=raw, in_=b_dram[None, :])
            if cdt == fp32:
                return raw
            r = const.tile([1, D_MODEL], cdt, tag=f"{name}rr")
            nc.vector.tensor_copy(r, raw)
            return r

        bv_sb = load_b_row(bv, "bv")
        bo_sb = load_b_row(bo, "bo")

        for b in range(BL):
            # ---- load x_b and build xT (rounded) ----
            x_sb = bigp.tile([P, QT_T, D_MODEL], fp32, tag="x_sb")
            nc.sync.dma_start(out=x_sb,
                              in_=x[b].rearrange("(t p) d -> p t d", p=P))
            if cdt == fp32:
                x_cd = x_sb
            else:
                x_cd = bigp.tile([P, QT_T, D_MODEL], cdt, tag="x_cd")
                nc.vector.tensor_copy(x_cd, x_sb)
            xT = bigp.tile([P, DIN_T, S], cdt, tag="xT")
            for dt_i in range(DIN_T):
                for qg in range(2):             # groups of 4 q-tiles
                    pst = ps_tr.tile([P, 512], fp32)
                    for j in range(4):
                        qt = qg * 4 + j
                        nc.tensor.transpose(
                            pst[:, j * P:(j + 1) * P],
                            x_cd[:, qt, dt_i * P:(dt_i + 1) * P],
                            ident)
                    nc.scalar.copy(
                        xT[:, dt_i, qg * 512:(qg + 1) * 512], pst)

            # ---- projections ----
            QT = bigp.tile([P, DIN_T, S], cdt, tag="QT")
            KT = bigp.tile([P, DIN_T, S], cdt, tag="KT")
            for W_sb, b_sb, dst in ((Wq_sb, bq_sb, QT), (Wk_sb, bk_sb, KT)):
                for ot in range(DIN_T):
                    for qc in range(2):
                        ps = ps_mm.tile([P, 512], fp32, tag="ps_s0", name=f"ps_p{ot}_{qc}")
                        for kt in range(DIN_T):
                            nc.tensor.matmul(
                                ps,
                                W_sb[:, kt, ot * P:(ot + 1) * P],
                                xT[:, kt, qc * 512:(qc + 1) * 512],
                                start=(kt == 0), stop=(kt == DIN_T - 1))
                        nc.scalar.activation(
                            dst[:, ot, qc * 512:(qc + 1) * 512], ps,
                            AF.Identity, bias=b_sb[:, ot:ot + 1])

            V = bigp.tile([P, QT_T, D_MODEL], cdt, tag="V")
            for tt in range(QT_T):
                ps = ps_mm.tile([P, 512], fp32, tag="ps_s1", name=f"ps_v{tt}")
                for kt in range(DIN_T):
                    nc.tensor.matmul(
                        ps,
                        xT[:, kt, tt * P:(tt + 1) * P],
                        Wv_sb[:, kt, :],
                        start=(kt == 0), stop=False)
                nc.tensor.matmul(ps, ones, bv_sb, start=False, stop=True)
                nc.scalar.copy(V[:, tt, :], ps)

            ctxT = bigp.tile([P, DIN_T, S], cdt, tag="ctxT")

            # ---- attention per head ----
            for h in range(N_HEADS):
                hp = 64 * (h % 2)
                ht = h // 2
                QT_h = QT[hp:hp + 64, ht, :]
                KT_h = KT[hp:hp + 64, ht, :]
                for qc in range(2):               # q chunks of 512
                    attnT = midp.tile([P, QT_T, 512], cdt, tag="attnT")
                    for j in range(4):            # q tiles of 128
                        qt = qc * 4 + j
                        pss = [ps_mm.tile([P, 512], fp32, tag=f"ps_s{i}",
                                          name=f"ps_s{i}_{b}_{h}_{qt}")
                               for i in range(KC)]
                        for kc in range(KC):
                            nc.tensor.matmul(
                                pss[kc],
                                QT_h[:, qt * P:(qt + 1) * P],
                                KT_h[:, kc * 512:(kc + 1) * 512],
                                start=True, stop=True)
                        E = rowp.tile([P, S], fp32, tag="E")
                        sums2 = smal.tile([P, KC], fp32, tag="sums2")
                        for kc in range(KC):
                            nc.scalar.activation(
                                E[:, kc * 512:(kc + 1) * 512], pss[kc],
                                AF.Exp, scale=SCALE,
                                accum_out=sums2[:, kc:kc + 1])
                        sums = smal.tile([P, 1], fp32, tag="sums")
                        nc.vector.tensor_reduce(
                            sums, sums2, axis=mybir.AxisListType.X,
                            op=mybir.AluOpType.add)
                        recip = smal.tile([P, 1], fp32, tag="recip")
                        nc.vector.reciprocal(recip, sums)
                        attn_sb = rowp.tile([P, S], cdt, tag="attn_sb")
                        nc.vector.tensor_scalar_mul(attn_sb, E, recip)
                        nc.sync.dma_start(
                            out=attn[b, h, qt * P:(qt + 1) * P, :],
                            in_=attn_sb)
                        # transpose attn tile into attnT[:, kt, j*128...]
                        for kg in range(2):       # groups of 4 k-tiles
                            pst = ps_tr.tile([P, 512], fp32)
                            for i in range(4):
                                kt = kg * 4 + i
                                nc.tensor.transpose(
                                    pst[:, i * P:(i + 1) * P],
                                    attn_sb[:, kt * P:(kt + 1) * P],
                                    ident)
                            nc.scalar.copy(
                                attnT[:, kg * 4:kg * 4 + 4, j * P:(j + 1) * P],
                                pst.rearrange("p (t q) -> p t q", q=P))
                    # ctx for this q-chunk, accumulate over k tiles
                    psc = ps_cx.tile([P, 512], fp32)
                    for kt in range(QT_T):
                        nc.tensor.matmul(
                            psc[0:64, :],
                            V[:, kt, h * 64:(h + 1) * 64],
                            attnT[:, kt, :],
                            start=(kt == 0), stop=(kt == QT_T - 1))
                    nc.scalar.copy(
                        ctxT[hp:hp + 64, ht, qc * 512:(qc + 1) * 512],
                        psc[0:64, :])

            # ---- output projection ----
            for qt in range(QT_T):
                ps = ps_mm.tile([P, 512], fp32, tag="ps_s0", name=f"ps_o{qt}")
                for dt_i in range(DIN_T):
                    nc.tensor.matmul(
                        ps,
                        ctxT[:, dt_i, qt * P:(qt + 1) * P],
                        Wo_sb[:, dt_i, :],
                        start=(dt_i == 0), stop=False)
                nc.tensor.matmul(ps, ones, bo_sb, start=False, stop=True)
                o_sb = outp.tile([P, D_MODEL], fp32, tag="o_sb")
                nc.scalar.copy(o_sb, ps)
                nc.sync.dma_start(out=out[b, qt * P:(qt + 1) * P, :], in_=o_sb)

    nc.compile()
    return nc


_CACHE = {}


def _get_nc():
    key = MM_DTYPE
    if key not in _CACHE:
        _CACHE[key] = _build()
    return _CACHE[key]


def kernel(x, Wq, bq, Wk, bk, Wv, bv, Wo, bo, trace=False, tmpdir=None):
    from concourse.bass_utils import run_bass_kernel_spmd

    nc = _get_nc()
    shared = {
        "Wq": np.ascontiguousarray(Wq, np.float32),
        "Wk": np.ascontiguousarray(Wk, np.float32),
        "Wv": np.ascontiguousarray(Wv, np.float32),
        "Wo": np.ascontiguousarray(Wo, np.float32),
        "bq": np.ascontiguousarray(bq, np.float32),
        "bk": np.ascontiguousarray(bk, np.float32),
        "bv": np.ascontiguousarray(bv, np.float32),
        "bo": np.ascontiguousarray(bo, np.float32),
    }
    x = np.ascontiguousarray(x, np.float32)
    in_maps = [dict(shared, x=x[c * BL:(c + 1) * BL]) for c in range(N_CORES)]
    res = run_bass_kernel_spmd(
        nc, in_maps, core_ids=list(range(N_CORES)), trace=trace,
        tmpdir=tmpdir)
    out = np.concatenate([res.results[c]["out"] for c in range(N_CORES)], axis=0)
    attn = np.concatenate([res.results[c]["attn"] for c in range(N_CORES)], axis=0)
    kernel.last_results = res
    return out, attn


# revision 12
# speedup vs baseline: 1.6500x; 1.6500x over previous
"""Multi-head attention Bass/Tile kernel for Trainium2, SPMD over 8 NeuronCores.

Problem: B=16, S=1024, D=512, H=8, dk=64.  out = MHA(x); returns
(out [B,S,D], attn_weights [B,H,S,S]).  Data-parallel over B: each of the 8
cores processes 2 batches end-to-end (no collectives needed).

Layout strategy per core (BL=2 local batches):
  xT   [din=128p, 4t, S]      <- PE-transpose of x_b
  QT/KT[dout=128p, 4t, S]     = W.T @ x.T   (head h lives at partitions
                                 64*(h%2) of tile h//2)
  V    [tok=128p, 8t, D]      = x @ Wv  (natural layout; lhsT for ctx matmul)
  per (b,h), per q-tile(128):
    scores psum [128q, 1024k] = QT_h.T @ KT_h   (K=dk=64)
    E = exp(scores/8)  (ACT, accum_out -> row sums)
    attn = E * recip(sums)   (DVE per-partition scalar) -> DMA out
    attnT [k=128p, 8t, 512q] <- PE-transpose of attn tiles
  per q-chunk(512): ctxT[dk,q] psum += V_h.T @ attnT   (K=k tiles)
  out[q, D] = sum_t ctxT_t.T @ Wo_t + bo  -> DMA out
"""

import os
import sys
import tempfile

import numpy as np

sys.path.insert(0, "/opt/trn_rl_repo")

B, S, D_MODEL, N_HEADS = 16, 1024, 512, 8
D_K = D_MODEL // N_HEADS        # 64
N_CORES = 8
BL = B // N_CORES               # 2 local batches per core
P = 128                         # partitions
DIN_T = D_MODEL // P            # 4 din tiles
QT_T = S // P                   # 8 q tiles per batch
KC = S // 512                   # 2 k chunks of 512
SCALE = 1.0 / float(np.sqrt(D_K))

# compute dtype for matmul operands: "float32" (exact, 4 cyc/row) or
# "float32r" (tf32-like, 1 cyc/row at free>=256)
MM_DTYPE = os.environ.get("MHA_MM_DTYPE", "float32r")


def _build(nc_holder=[]):
    import concourse.bass as bass
    import concourse.tile as tile
    from concourse import bacc, mybir
    from concourse.masks import make_identity

    fp32 = mybir.dt.float32
    cdt = getattr(mybir.dt, MM_DTYPE)
    AF = mybir.ActivationFunctionType

    nc = bacc.Bacc("TRN2", target_bir_lowering=False, debug=False,
                   num_devices=N_CORES)

    x = nc.dram_tensor("x", [BL, S, D_MODEL], fp32, kind="ExternalInput").ap()
    Wq = nc.dram_tensor("Wq", [D_MODEL, D_MODEL], fp32, kind="ExternalInput").ap()
    Wk = nc.dram_tensor("Wk", [D_MODEL, D_MODEL], fp32, kind="ExternalInput").ap()
    Wv = nc.dram_tensor("Wv", [D_MODEL, D_MODEL], fp32, kind="ExternalInput").ap()
    Wo = nc.dram_tensor("Wo", [D_MODEL, D_MODEL], fp32, kind="ExternalInput").ap()
    bq = nc.dram_tensor("bq", [D_MODEL], fp32, kind="ExternalInput").ap()
    bk = nc.dram_tensor("bk", [D_MODEL], fp32, kind="ExternalInput").ap()
    bv = nc.dram_tensor("bv", [D_MODEL], fp32, kind="ExternalInput").ap()
    bo = nc.dram_tensor("bo", [D_MODEL], fp32, kind="ExternalInput").ap()
    out = nc.dram_tensor("out", [BL, S, D_MODEL], fp32, kind="ExternalOutput").ap()
    attn = nc.dram_tensor("attn", [BL, N_HEADS, S, S], fp32,
                          kind="ExternalOutput").ap()

    from contextlib import ExitStack

    with tile.TileContext(nc) as tc, ExitStack() as ctx:
        const = ctx.enter_context(tc.tile_pool(name="const", bufs=1))
        bigp = ctx.enter_context(tc.tile_pool(name="bigp", bufs=1))   # per-batch 2MB tiles
        midp = ctx.enter_context(tc.tile_pool(name="midp", bufs=2))   # attnT ring
        rowp = ctx.enter_context(tc.tile_pool(name="rowp", bufs=3))   # E/attn rows
        smal = ctx.enter_context(tc.tile_pool(name="smal", bufs=4))   # stats
        outp = ctx.enter_context(tc.tile_pool(name="outp", bufs=2))
        # psum pools
        ps_mm = ctx.enter_context(tc.tile_pool(name="ps_mm", bufs=2, space="PSUM"))
        ps_tr = ctx.enter_context(tc.tile_pool(name="ps_tr", bufs=2, space="PSUM"))
        ps_cx = ctx.enter_context(tc.tile_pool(name="ps_cx", bufs=2, space="PSUM"))

        ident_f32 = const.tile([P, P], fp32)
        make_identity(nc, ident_f32)
        ident = const.tile([P, P], cdt)
        nc.vector.tensor_copy(ident, ident_f32)

        ones_f32 = const.tile([1, P], fp32)
        nc.vector.memset(ones_f32, 1.0)
        ones = const.tile([1, P], cdt)
        nc.vector.tensor_copy(ones, ones_f32)

        # weights in [din_part, din_tile, dout] layout, rounded to compute dtype
        def load_w(w_dram, name):
            raw = const.tile([P, DIN_T, D_MODEL], fp32, tag="wraw",
                             name=f"{name}_raw")
            nc.sync.dma_start(out=raw, in_=w_dram.rearrange("(t p) n -> p t n", p=P))
            r = const.tile([P, DIN_T, D_MODEL], cdt, tag=f"{name}r")
            nc.vector.tensor_copy(r, raw)
            return r

        Wq_sb = load_w(Wq, "wq")
        Wk_sb = load_w(Wk, "wk")
        Wv_sb = load_w(Wv, "wv")
        Wo_sb = load_w(Wo, "wo")

        # per-partition bias layout [128, 4] for Q/K (bias on dout partition dim)
        def load_b_part(b_dram, name):
            t = const.tile([P, DIN_T], fp32, tag=f"{name}p")
            nc.sync.dma_start(out=t, in_=b_dram.rearrange("(t p) -> p t", p=P))
            return t

        bq_sb = load_b_part(bq, "bq")
        bk_sb = load_b_part(bk, "bk")

        # row bias layout [1, 512] for V / out (bias on dout free dim)
        def load_b_row(b_dram, name):
            raw = const.tile([1, D_MODEL], fp32, tag=f"{name}rraw")
            nc.sync.dma_start(out=raw, in_=b_dram[None, :])
            if cdt == fp32:
                return raw
            r = const.tile([1, D_MODEL], cdt, tag=f"{name}rr")
            nc.vector.tensor_copy(r, raw)
            return r

        bv_sb = load_b_row(bv, "bv")
        bo_sb = load_b_row(bo, "bo")

        for b in range(BL):
            # ---- load x_b and build xT (rounded) ----
            x_sb = bigp.tile([P, QT_T, D_MODEL], fp32, tag="x_sb")
            nc.sync.dma_start(out=x_sb,
                              in_=x[b].rearrange("(t p) d -> p t d", p=P))
            xT = bigp.tile([P, DIN_T, S], cdt, tag="xT")
            for dt_i in range(DIN_T):
                for qg in range(2):             # groups of 4 q-tiles
                    pst = ps_tr.tile([P, 512], fp32, tag="pst", name="pst_x")
                    for j in range(4):
                        qt = qg * 4 + j
                        nc.tensor.transpose(
                            pst[:, j * P:(j + 1) * P],
                            x_sb[:, qt, dt_i * P:(dt_i + 1) * P],
                            ident_f32)
                    nc.scalar.copy(
                        xT[:, dt_i, qg * 512:(qg + 1) * 512], pst)

            # ---- projections ----
            QT = bigp.tile([P, DIN_T, S], cdt, tag="QT")
            KT = bigp.tile([P, DIN_T, S], cdt, tag="KT")
            for W_sb, b_sb, dst in ((Wq_sb, bq_sb, QT), (Wk_sb, bk_sb, KT)):
                for ot in range(DIN_T):
                    for qc in range(2):
                        ps = ps_mm.tile([P, 512], fp32, tag="ps_s0", name=f"ps_p{ot}_{qc}")
                        for kt in range(DIN_T):
                            nc.tensor.matmul(
                                ps,
                                W_sb[:, kt, ot * P:(ot + 1) * P],
                                xT[:, kt, qc * 512:(qc + 1) * 512],
                                start=(kt == 0), stop=(kt == DIN_T - 1))
                        nc.scalar.activation(
                            dst[:, ot, qc * 512:(qc + 1) * 512], ps,
                            AF.Identity, bias=b_sb[:, ot:ot + 1])

            V = bigp.tile([P, QT_T, D_MODEL], cdt, tag="V")
            for tt in range(QT_T):
                ps = ps_mm.tile([P, 512], fp32, tag="ps_s1", name=f"ps_v{tt}")
                for kt in range(DIN_T):
                    nc.tensor.matmul(
                        ps,
                        xT[:, kt, tt * P:(tt + 1) * P],
                        Wv_sb[:, kt, :],
                        start=(kt == 0), stop=False)
                nc.tensor.matmul(ps, ones, bv_sb, start=False, stop=True)
                nc.scalar.copy(V[:, tt, :], ps)

            ctxT = bigp.tile([P, DIN_T, S], cdt, tag="ctxT")

            # ---- attention per head ----
            for h in range(N_HEADS):
                hp = 64 * (h % 2)
                ht = h // 2
                QT_h = QT[hp:hp + 64, ht, :]
                KT_h = KT[hp:hp + 64, ht, :]
                for qc in range(2):               # q chunks of 512
                    attnT = midp.tile([P, QT_T, 512], cdt, tag="attnT")
                    for j in range(4):            # q tiles of 128
                        qt = qc * 4 + j
                        pss = [ps_mm.tile([P, 512], fp32, tag=f"ps_s{i}",
                                          name=f"ps_s{i}_{b}_{h}_{qt}")
                               for i in range(KC)]
                        for kc in range(KC):
                            nc.tensor.matmul(
                                pss[kc],
                                QT_h[:, qt * P:(qt + 1) * P],
                                KT_h[:, kc * 512:(kc + 1) * 512],
                                start=True, stop=True)
                        E = rowp.tile([P, S], fp32, tag="E")
                        sums2 = smal.tile([P, KC], fp32, tag="sums2")
                        for kc in range(KC):
                            nc.scalar.activation(
                                E[:, kc * 512:(kc + 1) * 512], pss[kc],
                                AF.Exp, scale=SCALE,
                                accum_out=sums2[:, kc:kc + 1])
                        sums = smal.tile([P, 1], fp32, tag="sums")
                        nc.vector.tensor_reduce(
                            sums, sums2, axis=mybir.AxisListType.X,
                            op=mybir.AluOpType.add)
                        recip = smal.tile([P, 1], fp32, tag="recip")
                        nc.vector.reciprocal(recip, sums)
                        attn_sb = rowp.tile([P, S], cdt, tag="attn_sb")
                        nc.vector.tensor_scalar_mul(attn_sb, E, recip)
                        nc.sync.dma_start(
                            out=attn[b, h, qt * P:(qt + 1) * P, :],
                            in_=attn_sb.bitcast(fp32))
                        # transpose attn tile into attnT[:, kt, j*128...]
                        for kg in range(2):       # groups of 4 k-tiles
                            pst = ps_tr.tile([P, 512], cdt, tag="pst", name="pst_a")
                            for i in range(4):
                                kt = kg * 4 + i
                                nc.tensor.transpose(
                                    pst[:, i * P:(i + 1) * P],
                                    attn_sb[:, kt * P:(kt + 1) * P],
                                    ident)
                            nc.scalar.copy(
                                attnT[:, kg * 4:kg * 4 + 4, j * P:(j + 1) * P],
                                pst.rearrange("p (t q) -> p t q", q=P))
                    # ctx for this q-chunk, accumulate over k tiles
                    psc = ps_cx.tile([P, 512], fp32)
                    for kt in range(QT_T):
                        nc.tensor.matmul(
                            psc[0:64, :],
                            V[:, kt, h * 64:(h + 1) * 64],
                            attnT[:, kt, :],
                            start=(kt == 0), stop=(kt == QT_T - 1))
                    nc.scalar.copy(
                        ctxT[hp:hp + 64, ht, qc * 512:(qc + 1) * 512],
                        psc[0:64, :])

            # ---- output projection ----
            for qt in range(QT_T):
                ps = ps_mm.tile([P, 512], fp32, tag="ps_s0", name=f"ps_o{qt}")
                for dt_i in range(DIN_T):
                    nc.tensor.matmul(
                        ps,
                        ctxT[:, dt_i, qt * P:(qt + 1) * P],
                        Wo_sb[:, dt_i, :],
                        start=(dt_i == 0), stop=False)
                nc.tensor.matmul(ps, ones, bo_sb, start=False, stop=True)
                o_sb = outp.tile([P, D_MODEL], fp32, tag="o_sb")
                nc.scalar.copy(o_sb, ps)
                nc.sync.dma_start(out=out[b, qt * P:(qt + 1) * P, :], in_=o_sb)

    nc.compile()
    return nc


_CACHE = {}


def _get_nc():
    key = MM_DTYPE
    if key not in _CACHE:
        _CACHE[key] = _build()
    return _CACHE[key]


def kernel(x, Wq, bq, Wk, bk, Wv, bv, Wo, bo, trace=False, tmpdir=None):
    from concourse.bass_utils import run_bass_kernel_spmd

    nc = _get_nc()
    shared = {
        "Wq": np.ascontiguousarray(Wq, np.float32),
        "Wk": np.ascontiguousarray(Wk, np.float32),
        "Wv": np.ascontiguousarray(Wv, np.float32),
        "Wo": np.ascontiguousarray(Wo, np.float32),
        "bq": np.ascontiguousarray(bq, np.float32),
        "bk": np.ascontiguousarray(bk, np.float32),
        "bv": np.ascontiguousarray(bv, np.float32),
        "bo": np.ascontiguousarray(bo, np.float32),
    }
    x = np.ascontiguousarray(x, np.float32)
    in_maps = [dict(shared, x=x[c * BL:(c + 1) * BL]) for c in range(N_CORES)]
    res = run_bass_kernel_spmd(
        nc, in_maps, core_ids=list(range(N_CORES)), trace=trace,
        tmpdir=tmpdir)
    out = np.concatenate([res.results[c]["out"] for c in range(N_CORES)], axis=0)
    attn = np.concatenate([res.results[c]["attn"] for c in range(N_CORES)], axis=0)
    kernel.last_results = res
    return out, attn
